# revision 27
# baseline (speedup 1.0000x reference)
"""Chebyshev encoder (T_0..T_29 per feature) as a Bass/Tile kernel on 8 NeuronCores.

Input  x  : [16, 65536, 2] f32
Output    : [16, 65536, 60] f32, out[..., d*30+n] = T_n(x[..., d])

Sharding: flatten B*N = 1,048,576 positions, split evenly across 8 cores
(131,072 positions each) — fully data parallel, no communication.

Per-core kernel: positions viewed as [128 partitions x 1024], processed in
chunks of C=128 positions per partition (8 tiles, triple-buffered output).
Orders are computed with forward-only blocked levels
(T_n = 2*T_L*T_{n-L} - T_{n-2L}), split across three engines: DVE does T_0
and all odd orders (two blocked stride-2 ops per level), ACT does T_1 and
Square(sqrt2*T_k) blocks for even orders, GPSIMD finishes evens with a
blocked "-1" tensor_scalar and issues loads. The SBUF output tile is laid
out exactly like the DRAM output (pos, d, n) so the store DMA is fully
contiguous per partition (30 KiB/partition per store). The kernel is
store-bandwidth-bound (~30 MiB written per core).

Hardware-learned constraints baked in here:
- engine/DMA instructions reliably encode only ONE sync wait ->
  split_multi_waits() post-pass legalizes anything wider;
- negative-stride DVE APs crash the device -> forward-only level scheme;
- per-semaphore counts must stay < ~128 in straight-line code.
"""

import dataclasses
import os

import numpy as np

import concourse.bass as bass
import concourse.tile as tile
from concourse import mybir
from concourse.bass_utils import run_bass_kernel_spmd

F32 = mybir.dt.float32
ALU = mybir.AluOpType

ORDER = 30
N_CORES = 8
B, N, D = 16, 65536, 2
POS_PER_CORE = B * N // N_CORES  # 131072
PPP = POS_PER_CORE // 128        # 1024 positions per partition
C = 128                          # positions per partition per tile
# Forward-only levels: prefix T_0..T_{p-1} known -> T_p..T_{p+L-1} via
# T_n = 2*T_L*T_{n-L} - T_{n-2L}. All slices ascend (the reversed-block
# variant's negative-stride APs crash the DVE on real hardware).
FWD_LEVELS = [(3, 1), (4, 2), (6, 3), (9, 4), (13, 6), (19, 9), (28, 2)]


def _flip_last(ap: bass.AP, n: int):
    """Reverse the last free dim of an AP (negative stride, offset at end)."""
    dims = [list(d) for d in ap.ap]
    stride, num = dims[-1]
    assert num == n
    new_offset = ap.offset + stride * (num - 1)
    dims[-1] = [-stride, num]
    return dataclasses.replace(ap, offset=new_offset, ap=dims)


def _substep(ap: bass.AP, start: int, count: int, step: int):
    """Last-free-dim subrange [start, start+step*count) with stride multiplier
    step (positive strides only — HW-safe)."""
    dims = [list(d) for d in ap.ap]
    stride, _num = dims[-1]
    dims[-1] = [stride * step, count]
    return dataclasses.replace(ap, offset=ap.offset + stride * start, ap=dims)


def split_multi_waits(nc):
    """Walrus codegen reliably encodes only ONE sync wait per instruction
    (spilling extra waits onto neighbors is flaky). Split any instruction
    carrying N>1 waits into N-1 single-wait drains followed by the original
    instruction with its last wait."""
    k = 0
    for fn in nc.m.functions:
        for blk in fn.blocks:
            out = []
            for ins in blk.instructions:
                si = ins.sync_info
                if si is not None and len(si.on_wait) > 1:
                    for w in si.on_wait[:-1]:
                        d = mybir.InstDrain(
                            name=f"wsplit_{k}",
                            sync_info=mybir.SyncInfo(on_wait=[w], on_update=[]),
                        )
                        d.engine = ins.engine
                        out.append(d)
                        k += 1
                    ins.sync_info = mybir.SyncInfo(
                        on_wait=[si.on_wait[-1]], on_update=si.on_update
                    )
                out.append(ins)
            blk.instructions[:] = out
    return k


def build_nc(split=True, C=C, xb=4, ob=3, ub=2, qb=2):
    nc = bass.Bass("TRN2", target_bir_lowering=False, debug=False)
    x_d = nc.dram_tensor("x", [POS_PER_CORE, D], F32, kind="ExternalInput")
    o_d = nc.dram_tensor("out", [POS_PER_CORE, D * ORDER], F32, kind="ExternalOutput")

    x_r = x_d.ap().rearrange("(p n) d -> p n d", p=128)
    o_r = o_d.ap().rearrange("(p n) (d o) -> p n d o", p=128, d=D)

    SQRT2 = float(np.sqrt(np.float32(2.0)))
    SQUARE = mybir.ActivationFunctionType.Square
    with tile.TileContext(nc) as tc:
        with (
            tc.tile_pool(name="xp", bufs=xb) as xp,
            tc.tile_pool(name="op", bufs=ob) as op,
            tc.tile_pool(name="up", bufs=ub) as up,
            tc.tile_pool(name="qp", bufs=qb) as qp,
        ):
            for t in range(PPP // C):
                xt = xp.tile([128, C, D], F32)
                nc.gpsimd.dma_start(xt[:], x_r[:, t * C:(t + 1) * C, :])

                ot = op.tile([128, C, D, ORDER], F32)
                u2 = up.tile([128, C, D, 5], F32)   # odd-order products (DVE)
                sq2 = qp.tile([128, C, D, 4], F32)  # 2*T_k^2 blocks (ACT)

                # Work split: DVE = T_0 + all odd orders (2 blocked stride-2
                # ops per level); ACT = T_1 copy + blocked Square(sqrt2*T_k)
                # (= 2*T_k^2) for even orders; GPSIMD = the "-1" affine
                # finishing evens (1-input op, line rate) + loads.
                nc.vector.memset(ot[:, :, :, 0], 1.0)            # T_0 = 1
                nc.scalar.copy(ot[:, :, :, 1], xt[:])            # T_1 = x
                nc.scalar.activation(sq2[:, :, :, 0], xt[:], SQUARE, scale=SQRT2)
                nc.gpsimd.tensor_scalar_sub(ot[:, :, :, 2], sq2[:, :, :, 0], 1.0)

                for p, L in FWD_LEVELS:
                    ns = range(p, p + L)
                    odds = [n for n in ns if n % 2 == 1]
                    evens = [n for n in ns if n % 2 == 0 and n > 2]
                    if odds:
                        o0, Lo = odds[0], len(odds)
                        u2s = u2[:, :, :, 0:Lo]
                        Tin = _substep(ot[:], o0 - L, Lo, 2)
                        Tk = ot[:, :, :, L:L + 1].broadcast_to([128, C, D, Lo])
                        # u2 = (T_{n-L} * 2) * T_L ; T_n = u2 - T_{n-2L}
                        nc.vector.scalar_tensor_tensor(
                            u2s, Tin, 2.0, Tk, ALU.mult, ALU.mult
                        )
                        nc.vector.tensor_tensor(
                            _substep(ot[:], o0, Lo, 2), u2s,
                            _substep(ot[:], o0 - 2 * L, Lo, 2), ALU.subtract,
                        )
                    if evens:
                        e0, Le = evens[0], len(evens)
                        k0 = e0 // 2
                        sqs = sq2[:, :, :, 0:Le]
                        # sq2 = 2*T_k^2 ; T_2k = sq2 - 1
                        nc.scalar.activation(
                            sqs, ot[:, :, :, k0:k0 + Le], SQUARE, scale=SQRT2
                        )
                        nc.gpsimd.tensor_scalar_sub(
                            _substep(ot[:], e0, Le, 2), sqs, 1.0
                        )

                nc.sync.dma_start(o_r[:, t * C:(t + 1) * C, :, :], ot[:])
    if split:
        split_multi_waits(nc)
    return nc


def run(x: np.ndarray, **spmd_kwargs):
    """Shard, execute on 8 cores, gather. Returns (output, BassKernelResults)."""
    x = np.ascontiguousarray(np.asarray(x, dtype=np.float32)).reshape(-1, D)
    assert x.shape[0] == N_CORES * POS_PER_CORE
    shards = x.reshape(N_CORES, POS_PER_CORE, D)

    nc = build_nc()
    in_maps = [{"x": np.ascontiguousarray(shards[i])} for i in range(N_CORES)]
    res = run_bass_kernel_spmd(nc, in_maps, list(range(N_CORES)), **spmd_kwargs)
    out = np.stack([res.results[i]["out"] for i in range(N_CORES)])
    return out.reshape(B, N, D * ORDER), res


def kernel(x: np.ndarray) -> np.ndarray:
    out, _ = run(x)
    return out
